# revision 3
# baseline (speedup 1.0000x reference)
"""4-layer LSTM encoder on 8 trn2 NeuronCores.

Strategy: data-parallel x2 over batch (B=64 -> 32/core-group) and
layer-pipeline x4 (core g*4+l owns layer l for batch half g).

Per core, per timestep, the full gate pre-activation
    gates = W_ih @ x_t + W_hh @ h_{t-1} + b           [4H, B] view
is computed as 16 K-tile matmuls with the *weights as the moving
operand* (batch=32 as the stationary operand, 4-way column-tiled PE),
accumulating 4 column-group partials in one PSUM bank.  A "transpose
reduce" matmul against a stacked-identity pattern then both sums the 4
partials and transposes the gates into [gate-dim-on-partitions, batch]
layout, where the LSTM cell (sigmoid/tanh on ScalarE, elementwise on
VectorE) runs and directly produces h^T, which is the stationary
operand for the next step.  c stays fp32; matmul operands are bf16.

Timesteps are processed in waves of C steps.  At the end of each wave
a 4-rank ring AllGather shares the wave's h^T chunk with the other
cores of the batch group; a core consumes its predecessor's chunk two
waves later (so the collective has 2 waves of compute to hide in).
The layer-l core runs 2*l garbage warmup waves (inputs zero, state
masked to zero via per-core 0/1 mask vectors) and captures its final
state with a one-hot per-core capture mask -- all cores run the exact
same program, only input data differs.
"""

import sys

sys.path.insert(0, "/opt/trn_rl_repo")

import numpy as np
import ml_dtypes

import concourse.bacc as bacc
import concourse.mybir as mybir
import concourse.tile as tile
from concourse.bass_utils import run_bass_kernel_spmd

F32 = mybir.dt.float32
BF16 = mybir.dt.bfloat16
AF = mybir.ActivationFunctionType
ALU = mybir.AluOpType

B, T, I, H, L = 64, 256, 512, 1024, 4
NSTEP = T - 1          # 255 real timesteps
BLOC = 32              # batch per core
NCHUNK = 8             # gate chunks of 512 (2 per gate type)
NKT = 16               # K tiles: 8 x-dims + 8 h-dims
G = 4                  # PE column-tile groups
C = 3                  # steps per wave
SKEW = 2               # consume AG from SKEW waves ago

_CACHE = {}


def _gate_perm():
    """maps packed gate column n (chunk-major) -> row of torch-order W.

    chunk c covers gate rows [tc*1024 + hf*512, +512) with tc=c//2,
    hf=c%2; within the chunk order is linear.
    """
    n = np.arange(4 * H)
    c = n // 512
    ni = n % 512
    return (c // 2) * H + (c % 2) * 512 + ni


def prep_core_inputs(core_id, inputs, nstep=NSTEP, c_steps=C):
    g, l = core_id // 4, core_id % 4
    perm = _gate_perm()
    nw = nstep // c_steps
    nwt = nw + SKEW * (L - 1)

    if l == 0:
        W_ih = np.asarray(inputs["W_ih0"])          # [4H, I]
        W_hh = np.asarray(inputs["W_hh0"])
        bias = np.asarray(inputs["b_ih0"]) + np.asarray(inputs["b_hh0"])
    else:
        W_ih = np.asarray(inputs["W_ih_rest"][l - 1])  # [4H, H]
        W_hh = np.asarray(inputs["W_hh_rest"][l - 1])
        bias = np.asarray(inputs["b_ih_rest"][l - 1]) + np.asarray(
            inputs["b_hh_rest"][l - 1]
        )

    # moving-operand weights: wmov[q, k, n] ; q<8 x-side, q>=8 h-side
    wmov = np.zeros((NKT, 128, 4 * H), np.float32)
    Wp_ih = W_ih[perm]  # [4H(packed), in_dim]
    Wp_hh = W_hh[perm]
    in_dim = Wp_ih.shape[1]
    for q in range(8):
        lo = q * 128
        if lo < in_dim:
            wmov[q] = Wp_ih[:, lo : lo + 128].T
    for q in range(8):
        wmov[8 + q] = Wp_hh[:, q * 128 : (q + 1) * 128].T
    wmov = wmov.reshape(NKT * 128, 4 * H).astype(ml_dtypes.bfloat16)

    # static input sequence, transposed: xstat[q,k,t,b]
    xstat = np.zeros((8, 128, nwt * c_steps, BLOC), np.float32)
    if l == 0:
        xb = np.asarray(inputs["batch"])[g * BLOC : (g + 1) * BLOC, 1 : nstep + 1, :]
        # [32, nstep, 512] -> [512, nstep, 32]
        xt = xb.transpose(2, 1, 0)
        for q in range(4):
            xstat[q, :, :nstep, :] = xt[q * 128 : (q + 1) * 128]
    xstat = xstat.reshape(8 * 128, nwt * c_steps * BLOC).astype(ml_dtypes.bfloat16)

    # bias in transposed layout, replicated over batch: [128, 4*8*32]
    biasrep = np.zeros((128, 4, 8, BLOC), np.float32)
    bp = bias[perm].reshape(NCHUNK, 4, 128)  # [chunk, j, p]
    for c in range(NCHUNK):
        tc, hf = c // 2, c % 2
        for j in range(4):
            biasrep[:, tc, hf * 4 + j, :] = bp[c, j][:, None]
    biasrep = biasrep.reshape(128, 4 * 8 * BLOC)

    # transpose-reduce pattern: 4 stacked 32x32 identities
    ones = np.zeros((128, BLOC), np.float32)
    ones[np.arange(128), np.arange(128) % BLOC] = 1.0
    ones = ones.astype(ml_dtypes.bfloat16)

    # per-core masks
    sel = np.zeros((128, 4), np.float32)
    if l > 0:
        sel[:, l - 1] = 1.0
    hmask = np.zeros((128, nwt), np.float32)
    k0 = SKEW * l
    hmask[:, k0 : k0 + nw] = 1.0
    capmask = np.zeros((128, nwt), np.float32)
    capmask[:, k0 + nw - 1] = 1.0

    return {
        "wmov": wmov,
        "xstat": xstat,
        "biasrep": biasrep,
        "tr_ones": ones,
        "sel": sel,
        "hmask": hmask,
        "capmask": capmask,
    }


def build_nc(nstep=NSTEP, c_steps=C, g_groups=G):
    nw = nstep // c_steps
    nwt = nw + SKEW * (L - 1)
    NR = (NKT + g_groups - 1) // g_groups
    nc = bacc.Bacc("TRN2", target_bir_lowering=False, debug=False, num_devices=8)

    wmov_d = nc.dram_tensor("wmov", [NKT * 128, 4 * H], BF16, kind="ExternalInput")
    xstat_d = nc.dram_tensor(
        "xstat", [8 * 128, nwt * c_steps * BLOC], BF16, kind="ExternalInput"
    )
    bias_d = nc.dram_tensor("biasrep", [128, 4 * 8 * BLOC], F32, kind="ExternalInput")
    ones_d = nc.dram_tensor("tr_ones", [128, BLOC], BF16, kind="ExternalInput")
    sel_d = nc.dram_tensor("sel", [128, 4], F32, kind="ExternalInput")
    hmask_d = nc.dram_tensor("hmask", [128, nwt], F32, kind="ExternalInput")
    capmask_d = nc.dram_tensor("capmask", [128, nwt], F32, kind="ExternalInput")
    hT_d = nc.dram_tensor("hT_out", [128, 8 * BLOC], F32, kind="ExternalOutput")
    cT_d = nc.dram_tensor("cT_out", [128, 8 * BLOC], F32, kind="ExternalOutput")

    CH = c_steps * BLOC  # free size of one (q) row-chunk per wave

    with tile.TileContext(nc) as tc:
        with (
            tc.tile_pool(name="wp", bufs=1) as wp,
            tc.tile_pool(name="const", bufs=1) as constp,
            tc.tile_pool(name="state", bufs=1) as statep,
            tc.tile_pool(name="xs", bufs=2) as xsp,
            tc.tile_pool(name="sh", bufs=2) as shp,
            tc.tile_pool(name="hstag", bufs=2) as hstagp,
            tc.tile_pool(name="work", bufs=3) as workp,
            tc.tile_pool(name="acts", bufs=2) as actp,
            tc.tile_pool(name="pspart", bufs=2, space="PSUM") as pspart,
            tc.tile_pool(name="psT", bufs=2, space="PSUM") as psTp,
            tc.tile_pool(name="dram", bufs=3, space="DRAM") as dramp,
        ):
            # ---- static loads ----
            wt = wp.tile([128, NKT, NCHUNK, 512], BF16, name="wt")
            nc.sync.dma_start(
                wt[:],
                wmov_d.rearrange("(q k) (c n) -> k q c n", k=128, n=512),
            )
            biasrep = constp.tile([128, 4, 8, BLOC], F32, name="biasrep")
            nc.sync.dma_start(
                biasrep[:], bias_d.rearrange("p (t s b) -> p t s b", t=4, b=BLOC)
            )
            ones_t = constp.tile([128, BLOC], BF16, name="ones_t")
            nc.sync.dma_start(ones_t[:], ones_d[:])
            sel_t = constp.tile([128, 4], F32, name="sel_t")
            nc.sync.dma_start(sel_t[:], sel_d[:])
            hmask_t = constp.tile([128, nwt], F32, name="hmask_t")
            nc.sync.dma_start(hmask_t[:], hmask_d[:])
            capmask_t = constp.tile([128, nwt], F32, name="capmask_t")
            nc.sync.dma_start(capmask_t[:], capmask_d[:])

            # ---- state ----
            c_state = [
                statep.tile([128, 8, BLOC], F32, name=f"c_state{i}") for i in range(2)
            ]
            nc.vector.memset(c_state[0][:], 0.0)
            nc.vector.memset(c_state[1][:], 0.0)
            hacc = [statep.tile([128, 8, BLOC], F32, name=f"hacc{i}") for i in range(2)]
            cacc = [statep.tile([128, 8, BLOC], F32, name=f"cacc{i}") for i in range(2)]
            nc.vector.memset(hacc[0][:], 0.0)
            nc.vector.memset(cacc[0][:], 0.0)
            hstag_init = statep.tile([128, 8, c_steps, BLOC], BF16, name="hstag_init")
            nc.vector.memset(hstag_init[:], 0.0)

            xstat_r = xstat_d.rearrange(
                "(q k) (t b) -> k q t b", k=128, b=BLOC
            )  # [128, 8, nwt*C, 32]

            prev_hstag = hstag_init
            ag_out = {}  # wave -> dram tile holding gathered chunks
            gstep = 0  # global step counter (for c-state parity)

            for w in range(nwt):
                # -- load this wave's static input chunk --
                xs = xsp.tile([128, 8, c_steps, BLOC], BF16, name="xs", tag="xs")
                nc.sync.dma_start(
                    xs[:], xstat_r[:, :, w * c_steps : (w + 1) * c_steps, :]
                )

                # -- fold in gathered h chunks from SKEW waves ago --
                if (w - SKEW) in ag_out:
                    src = ag_out.pop(w - SKEW)
                    xt = xs
                    for r in range(4):
                        sh = shp.tile(
                            [128, 8, c_steps, BLOC],
                            BF16,
                            name=f"sh{r}",
                            tag=f"sh{r % 2}",
                        )
                        nc.sync.dma_start(
                            sh[:],
                            src[r * 8 * 128 : (r + 1) * 8 * 128, :].rearrange(
                                "(q k) (t b) -> k q t b", k=128, b=BLOC
                            ),
                        )
                        xnew = xsp.tile(
                            [128, 8, c_steps, BLOC], BF16, name="xt", tag="xs"
                        )
                        nc.vector.scalar_tensor_tensor(
                            xnew[:], sh[:], sel_t[:, r : r + 1], xt[:],
                            ALU.mult, ALU.add,
                        )
                        xt = xnew
                    xuse = xt
                else:
                    xuse = xs

                hstag = hstagp.tile(
                    [128, 8, c_steps, BLOC], BF16, name="hstag", tag="hstag"
                )

                for s in range(c_steps):
                    par = gstep & 1
                    gstep += 1

                    def stat_slice(q, s=s, xuse=xuse, hstag=hstag,
                                   prev_hstag=prev_hstag):
                        if q < 8:
                            return xuse[:, q, s, :]
                        if s == 0:
                            return prev_hstag[:, q - 8, c_steps - 1, :]
                        return hstag[:, q - 8, s - 1, :]

                    psT = psTp.tile([128, 4, 8, BLOC], F32, name="psT", tag="psT")
                    for ch in range(NCHUNK):
                        tcg, hf = ch // 2, ch % 2
                        ps = pspart.tile([128, 512], F32, name="part", tag="part")
                        for q in range(NKT):
                            j = q % g_groups
                            r = q // g_groups
                            nc.tensor.matmul(
                                ps[32 * j : 32 * j + 32, :],
                                stat_slice(q),
                                wt[:, q, ch, :],
                                start=(r == 0),
                                stop=(r == NR - 1),
                                tile_position=(0, 32 * j),
                            )
                        pc = workp.tile([128, 512], BF16, name="pc", tag="pc")
                        if ch % 2 == 0:
                            nc.scalar.copy(pc[:], ps[:])
                        else:
                            nc.vector.tensor_copy(pc[:], ps[:])
                        for j in range(4):
                            nc.tensor.matmul(
                                psT[:, tcg, hf * 4 + j, :],
                                pc[:, 128 * j : 128 * (j + 1)],
                                ones_t[:],
                                start=True,
                                stop=True,
                            )

                    # ---- cell ----
                    act = []
                    for t in range(4):
                        pre = workp.tile(
                            [128, 8, BLOC], F32, name=f"pre{t}", tag=f"pre{t % 2}"
                        )
                        nc.vector.tensor_add(pre[:], psT[:, t, :, :], biasrep[:, t, :, :])
                        a = actp.tile([128, 8, BLOC], F32, name=f"act{t}", tag=f"act{t}")
                        nc.scalar.activation(
                            a[:], pre[:], AF.Tanh if t == 2 else AF.Sigmoid
                        )
                        act.append(a)
                    hm = hmask_t[:, w : w + 1]
                    t1 = workp.tile([128, 8, BLOC], F32, name="t1", tag="t1")
                    nc.vector.scalar_tensor_tensor(
                        t1[:], act[0][:], hm, act[2][:], ALU.mult, ALU.mult
                    )
                    t2 = workp.tile([128, 8, BLOC], F32, name="t2", tag="t2")
                    nc.vector.scalar_tensor_tensor(
                        t2[:], act[1][:], hm, c_state[par][:], ALU.mult, ALU.mult
                    )
                    nc.vector.tensor_add(c_state[1 - par][:], t1[:], t2[:])
                    tcn = workp.tile([128, 8, BLOC], F32, name="tcn", tag="tcn")
                    nc.scalar.activation(tcn[:], c_state[1 - par][:], AF.Tanh)
                    nc.vector.scalar_tensor_tensor(
                        hstag[:, :, s, :], act[3][:], hm, tcn[:], ALU.mult, ALU.mult
                    )

                # ---- wave epilogue: capture + share ----
                wpar = w & 1
                cm = capmask_t[:, w : w + 1]
                nc.vector.scalar_tensor_tensor(
                    hacc[1 - wpar][:],
                    hstag[:, :, c_steps - 1, :],
                    cm,
                    hacc[wpar][:],
                    ALU.mult,
                    ALU.add,
                )
                nc.vector.scalar_tensor_tensor(
                    cacc[1 - wpar][:],
                    c_state[gstep & 1][:],
                    cm,
                    cacc[wpar][:],
                    ALU.mult,
                    ALU.add,
                )

                if w < nwt - SKEW:
                    send = dramp.tile([8 * 128, CH], BF16, name="send", tag="send")
                    nc.sync.dma_start(
                        send.rearrange("(q k) f -> k q f", k=128),
                        hstag.rearrange("k q t b -> k q (t b)"),
                    )
                    agout = dramp.tile(
                        [4 * 8 * 128, CH], BF16, name="agout", tag="agout"
                    )
                    nc.gpsimd.collective_compute(
                        "AllGather",
                        ALU.bypass,
                        ins=[send.opt()],
                        outs=[agout.opt()],
                        replica_groups=[[0, 1, 2, 3], [4, 5, 6, 7]],
                    )
                    ag_out[w] = agout

                prev_hstag = hstag

            fpar = nwt & 1
            nc.sync.dma_start(
                hT_d.rearrange("p (s b) -> p s b", b=BLOC), hacc[fpar][:]
            )
            nc.sync.dma_start(
                cT_d.rearrange("p (s b) -> p s b", b=BLOC), cacc[fpar][:]
            )

    nc.compile()
    return nc


def _get_nc(nstep, c_steps, g_groups):
    key = (nstep, c_steps, g_groups)
    if key not in _CACHE:
        _CACHE[key] = build_nc(nstep, c_steps, g_groups)
    return _CACHE[key]


def run(inputs, nstep=NSTEP, c_steps=C, g_groups=G, **kw):
    nc = _get_nc(nstep, c_steps, g_groups)
    in_maps = [prep_core_inputs(cid, inputs, nstep, c_steps) for cid in range(8)]
    res = run_bass_kernel_spmd(nc, in_maps, core_ids=list(range(8)), **kw)

    h_final = np.zeros((L, B, H), np.float32)
    c_final = np.zeros((L, B, H), np.float32)
    for cid in range(8):
        g, l = cid // 4, cid % 4
        hT = np.asarray(res.results[cid]["hT_out"]).reshape(128, 8, BLOC)
        cT = np.asarray(res.results[cid]["cT_out"]).reshape(128, 8, BLOC)
        # value [p, s, b] = state[h-dim s*128+p, batch b]
        h_final[l, g * BLOC : (g + 1) * BLOC, :] = hT.transpose(2, 1, 0).reshape(
            BLOC, H
        )
        c_final[l, g * BLOC : (g + 1) * BLOC, :] = cT.transpose(2, 1, 0).reshape(
            BLOC, H
        )
    return h_final, c_final


def kernel(**inputs):
    return run(inputs)
